# revision 5
# baseline (speedup 1.0000x reference)
"""Trainium2 Bass kernel for NeuralGraphOutput (gnn_message_passing).

Math (per sample b):
    out[b, :] = sum_a mask[b,a] * relu(cat(atoms[b,a,:], sum_d bonds[b,a,d,:]) @ W + bias)
    mask[b,a] = any(edges[b,a,:] != -1)

Strategy: pure data parallel over 8 NeuronCores (256 samples each).
Tolerance is rel_err < 2e-2, so the host casts atoms/bonds/W to bf16 and
builds one fused row tensor per core:

    fused[row, 0:64]   = atoms        (bf16)
    fused[row, 64:80]  = 0            (bond-sum landing slots)
    fused[row, 80]     = 1            (bias column)
    fused[row, 81]     = 0            (pad -> KC=82 even)
    fused[row, 82:210] = bonds d-major (bf16)

rows are permuted host-side to [chunk, partition, sub, 210] so each
partition's per-chunk DMA source is one contiguous 3360B run (>=512B
avoids the DMA read-modify-write penalty) and each 1024-row chunk is a
single DMA instruction.

Per chunk (CH=8 sub-tiles of 128 rows):
  - DMA fused rows -> fused_sb  [128, 8*210]
  - Pool sums bonds D=8 -> fused_sb cols 64:80 (one tensor_reduce)
  - PE transposes fused_sb[:, c, 0:82] (bf16, 1 cyc/row) -> psum_ct
  - DVE drains psum_ct -> catT (bf16, 2x_1P mode)
  - main matmuls (bf16): psum_fp[128, 4*256] = catT.T @ W_aug, 2 half-chunks
  - ScalarE relu psum_fp -> relu16 (bf16)
  - reduction matmuls (bf16): psum_out[16,256] += mask16[:,t,:].T @ relu16
    (mask-weighted per-sample atom sum; masks precomputed on device from
     edges, laid out as one-hot columns per sub-tile)
  - after 16 samples (4 chunks) accumulate, DVE copies psum_out -> stage,
    SWDGE drains stage -> DRAM.

Instruction sync-wait budget is 1 inline semaphore wait; legalize_waits
splits any surplus into standalone EventSemaphore instructions.
"""

import os
from contextlib import ExitStack

import numpy as np

import concourse.bass as bass
import concourse.mybir as mybir
import concourse.tile as tile
from concourse import masks
from concourse.bass_utils import run_bass_kernel_spmd

# Problem shapes (hardcoded per contract)
B, A, D, FA, FB, FP = 2048, 256, 8, 64, 16, 256
NCORES = 8
P = 128
CH = 8                     # sub-tiles per chunk (1024 rows)
G = 4                      # sub-tiles per psum_fp half-chunk
KC = 82                    # contract rows: 64 atoms + 16 bond sums + bias + pad
FW = KC + D * FB           # fused row width: 82 + 128 = 210
SPG = 16                   # samples accumulated per psum_out group

f32 = mybir.dt.float32
bf16 = mybir.dt.bfloat16
i32 = mybir.dt.int32

np_bf16 = mybir.dt.np(bf16)

# Set by kernel() after a run; test.py reads exec_time_ns / trace info.
LAST_RESULTS = None


def legalize_waits(nc, max_inline=1):
    """This toolchain's walrus accepts at most one semaphore wait inline per
    instruction (64B Events struct). Tile emits multi-wait sync_info; split
    the surplus into standalone EventSemaphore instructions just before the
    instruction on the same engine queue — identical semantics."""
    f = nc.m.functions[0]
    for bb in f.blocks:
        new = []
        for inst in bb.instructions:
            si = inst.sync_info
            waits = list(si.on_wait) if (si and si.on_wait) else []
            if len(waits) > max_inline:
                keep = waits[-max_inline:]
                moved = waits[:-max_inline]
                for k, w in enumerate(moved):
                    new.append(
                        mybir.InstEventSemaphore(
                            name=f"{inst.name}-prewait{k}",
                            ins=[],
                            outs=[],
                            sync_info=mybir.SyncInfo(on_wait=[w], on_update=[]),
                            engine=inst.engine,
                        )
                    )
                si.on_wait = keep
            new.append(inst)
        bb.instructions[:] = new


def build_nc(n_samples_per_core: int, legalize: bool = True) -> bass.Bass:
    """Build the single-core Bass program (same program runs SPMD on all cores)."""
    BC = n_samples_per_core
    N = BC * A                      # flat rows per core
    NT = N // P                     # sub-tiles
    NCH = N // (CH * P)             # chunks
    CH_PER_GROUP = SPG * A // (CH * P)   # chunks per psum_out group (4)
    assert NCH % CH_PER_GROUP == 0

    nc = bass.Bass()
    fused_d = nc.dram_tensor("fused", [NCH * P, CH * FW], bf16, kind="ExternalInput")
    edges_d = nc.dram_tensor("edges", [N, D], i32, kind="ExternalInput")
    # host passes W stacked: rows 0:80 = W, row 80 = b, row 81 = 0 (pad)
    w_d = nc.dram_tensor("w", [KC, FP], bf16, kind="ExternalInput")
    out_d = nc.dram_tensor("out", [BC, FP], f32, kind="ExternalOutput")

    with ExitStack() as ctx:
        tc = ctx.enter_context(tile.TileContext(nc))
        singles = ctx.enter_context(tc.tile_pool(name="singles", bufs=1))

        # ---- constants ----
        w_sb = singles.tile([KC, FP], bf16)
        nc.sync.dma_start(out=w_sb[:], in_=w_d[:, :])
        # identity built on gpsimd, then laundered through DVE so consumers
        # depend on a single engine lane
        identity_src = singles.tile([P, P], bf16)
        masks.make_identity(nc, identity_src[:])
        identity = singles.tile([P, P], bf16)
        nc.vector.tensor_copy(identity[:], identity_src[:])

        # mask16[:, t, s] = mask of row t*128+p if sub-tile t belongs to
        # sample slot s of its 16-sample group, else 0.
        mask16 = singles.tile([P, NT, SPG], bf16)
        nc.vector.memset(mask16[:], 0.0)

        # PSUM pool for transposes — shared (same tag) between the prepass
        # and the main loop so slot reuse is PE-internal
        psct = ctx.enter_context(tc.tile_pool(name="psct", bufs=2, space="PSUM"))

        # ---- mask pre-pass (pool stays alive: avoids release-zone deps) ----
        RPP = N // P  # rows per partition
        pp = ctx.enter_context(tc.tile_pool(name="prepass", bufs=1))
        if True:
            edges_sb = pp.tile([P, RPP * D], i32)
            nc.sync.dma_start(
                out=edges_sb[:],
                in_=edges_d[:, :].rearrange("(p r) d -> p (r d)", p=P),
            )
            degmax = pp.tile([P, RPP], i32)
            nc.vector.tensor_reduce(
                out=degmax[:],
                in_=edges_sb.rearrange("p (r d) -> p r d", d=D),
                axis=mybir.AxisListType.X,
                op=mybir.AluOpType.max,
            )
            # mask = (max_d edge >= 0) as 1.0/0.0
            masknat = pp.tile([P, RPP], bf16)
            nc.vector.tensor_scalar(
                out=masknat[:], in0=degmax[:], scalar1=0, scalar2=None,
                op0=mybir.AluOpType.is_ge,
            )
            # maskT[:, t] = masks of rows [t*128, t*128+128)
            maskT = pp.tile([P, NT], bf16)
            nblk = RPP // P  # 4 column-blocks
            maskT_v = maskT.rearrange("p (c j) -> p c j", j=nblk)
            for j in range(nblk):
                pst = psct.tile([P, P], bf16, name="pst", tag="psum_ct")
                nc.tensor.transpose(
                    pst[:], masknat[:, j * P : (j + 1) * P], identity[:]
                )
                nc.vector.tensor_copy(maskT_v[:, :, j], pst[:])
            # scatter maskT columns into one-hot-by-sample-slot layout:
            # sub-tile t = 32u + 2s + h -> mask16 flat col 512u + 33s + 16h
            m16flat = mask16.rearrange("p t s -> p (t s)")
            maskT_w = maskT.rearrange("p (u w) -> p u w", w=2 * SPG)
            for s in range(SPG):
                for h in range(2):
                    dst = m16flat[:, 33 * s + SPG * h :: P * G]
                    nc.vector.tensor_copy(dst, maskT_w[:, :, 2 * s + h])

        # ---- main loop ----
        fusedp = ctx.enter_context(tc.tile_pool(name="fusedp", bufs=3))
        bs4p = ctx.enter_context(tc.tile_pool(name="bs4p", bufs=2))
        bs2p = ctx.enter_context(tc.tile_pool(name="bs2p", bufs=2))
        catTp = ctx.enter_context(tc.tile_pool(name="catTp", bufs=3))
        relup = ctx.enter_context(tc.tile_pool(name="relup", bufs=4))
        psfp = ctx.enter_context(tc.tile_pool(name="psfp", bufs=2, space="PSUM"))
        psout = ctx.enter_context(tc.tile_pool(name="psout", bufs=2, space="PSUM"))
        stagep = ctx.enter_context(tc.tile_pool(name="stagep", bufs=2))

        fused_r = fused_d[:, :].rearrange("(T p) f -> T p f", p=P)

        psum_out = None
        for T in range(NCH):
            fused_sb = fusedp.tile([P, CH * FW], bf16)
            nc.sync.dma_start(out=fused_sb[:], in_=fused_r[T])
            fv = fused_sb.rearrange("p (c f) -> p c f", f=FW)

            # bond sum over D: Pool fold ladder 8->4->2->1 (free-axis
            # tensor_reduce is DVE-only; Pool only has elementwise)
            bview = fv[:, :, KC:FW].rearrange("p c (e x) -> p c e x", e=2)
            bs4 = bs4p.tile([P, CH, (D // 2) * FB], bf16)
            nc.gpsimd.tensor_tensor(
                out=bs4[:], in0=bview[:, :, 0], in1=bview[:, :, 1],
                op=mybir.AluOpType.add,
            )
            b4view = bs4.rearrange("p c (e x) -> p c e x", e=2)
            bs2 = bs2p.tile([P, CH, (D // 4) * FB], bf16)
            nc.gpsimd.tensor_tensor(
                out=bs2[:], in0=b4view[:, :, 0], in1=b4view[:, :, 1],
                op=mybir.AluOpType.add,
            )
            b2view = bs2.rearrange("p c (e x) -> p c e x", e=2)
            nc.gpsimd.tensor_tensor(
                out=fv[:, :, FA : FA + FB], in0=b2view[:, :, 0], in1=b2view[:, :, 1],
                op=mybir.AluOpType.add,
            )

            # feature-major transpose via PE (bf16, 1 cyc/row)
            psum_ct = psct.tile([KC, CH * P], bf16, tag="psum_ct")
            for c in range(CH):
                nc.tensor.transpose(
                    psum_ct[:, c * P : (c + 1) * P],
                    fv[:, c, 0:KC],
                    identity[:],
                )
            catT = catTp.tile([KC, CH * P], bf16)
            nc.vector.tensor_scalar(
                out=catT[:], in0=psum_ct[:, :],
                scalar1=0.0, scalar2=None, op0=mybir.AluOpType.add,
            )

            for h2 in range(CH // G):
                # main matmuls (bias folded in via ones col 80)
                psum_fp = psfp.tile([P, G * FP], f32)
                for g in range(G):
                    c = h2 * G + g
                    nc.tensor.matmul(
                        psum_fp[:, g * FP : (g + 1) * FP],
                        lhsT=catT[:, c * P : (c + 1) * P],
                        rhs=w_sb[:, :],
                        start=True,
                        stop=True,
                    )

                relu = relup.tile([P, G * FP], bf16)
                nc.scalar.activation(
                    relu[:], psum_fp[:], mybir.ActivationFunctionType.Relu
                )

                # mask-weighted atom reduction
                for g in range(G):
                    t = CH * T + h2 * G + g
                    if t % (2 * SPG) == 0:
                        psum_out = psout.tile([SPG, FP], f32, name="psum_out")
                    nc.tensor.matmul(
                        psum_out[:, :],
                        lhsT=mask16[:, t, :],
                        rhs=relu[:, g * FP : (g + 1) * FP],
                        start=(t % (2 * SPG) == 0),
                        stop=(t % (2 * SPG) == 2 * SPG - 1),
                    )
                    if t % (2 * SPG) == 2 * SPG - 1:
                        grp = t // (2 * SPG)
                        stage = stagep.tile([SPG, FP], f32)
                        nc.vector.tensor_scalar(
                            out=stage[:], in0=psum_out[:],
                            scalar1=0.0, scalar2=None, op0=mybir.AluOpType.add,
                        )
                        # SWDGE so the output drain doesn't perturb the HWDGE
                        # lane rotation the input stream relies on
                        nc.gpsimd.dma_start(
                            out=out_d[grp * SPG : (grp + 1) * SPG, :], in_=stage[:]
                        )
    if legalize:
        legalize_waits(nc)
    return nc


def stack_w(W, b):
    """Host-side W layout matching catT rows: W | bias | zero pad (bf16)."""
    return np.ascontiguousarray(
        np.vstack(
            [
                np.asarray(W, dtype=np.float32),
                np.asarray(b, dtype=np.float32).reshape(1, FP),
                np.zeros((1, FP), dtype=np.float32),
            ]
        ).astype(np_bf16)
    )


def make_fused(atoms_flat, bonds_flat):
    """Build the fused [N, 210] bf16 tensor and permute to the DMA layout
    [NCH*P, CH*FW] where each partition's chunk data is contiguous.

    atoms_flat: [N, 64] f32/bf16, bonds_flat: [N, 128] f32/bf16.
    Row order: flat row = T*CH*P + c*P + p  ->  dram[(T, p), (c, f)].
    """
    N = atoms_flat.shape[0]
    NCH = N // (CH * P)
    fused = np.zeros((N, FW), dtype=np_bf16)
    fused[:, 0:FA] = atoms_flat.astype(np_bf16)
    fused[:, FA + FB] = 1.0
    fused[:, KC:FW] = bonds_flat.astype(np_bf16)
    perm = fused.reshape(NCH, CH, P, FW).transpose(0, 2, 1, 3)
    return np.ascontiguousarray(perm.reshape(NCH * P, CH * FW))


def _shard_inputs(atoms, bonds, edges, W, b, n_samples_per_core):
    BC = n_samples_per_core
    N = BC * A
    in_maps = []
    w_np = stack_w(W, b)
    atoms = np.asarray(atoms, dtype=np.float32)
    bonds = np.asarray(bonds, dtype=np.float32)
    edges = np.asarray(edges, dtype=np.int32)
    for c in range(NCORES):
        sl = slice(c * BC, (c + 1) * BC)
        in_maps.append(
            {
                "fused": make_fused(
                    atoms[sl].reshape(N, FA), bonds[sl].reshape(N, D * FB)
                ),
                "edges": np.ascontiguousarray(edges[sl].reshape(N, D)),
                "w": w_np,
            }
        )
    return in_maps


def kernel(atoms, bonds, edges, W, b):
    """Full inputs in, full output out. Shards batch across 8 cores."""
    global LAST_RESULTS
    BC = B // NCORES
    nc = build_nc(BC)
    in_maps = _shard_inputs(atoms, bonds, edges, W, b, BC)
    core_ids = list(range(NCORES))
    trace = bool(os.environ.get("KERNEL_TRACE"))
    res = run_bass_kernel_spmd(nc, in_maps, core_ids, trace=trace)
    LAST_RESULTS = res
    out = np.concatenate([res.results[c]["out"] for c in range(NCORES)], axis=0)
    return out.astype(np.float32)
